# revision 3
# baseline (speedup 1.0000x reference)
"""Causal self-attention on 8 NeuronCores (Bass/Tile, fp32r matmuls).

Sharding: tensor-parallel over heads x data-parallel over batch.
  core c -> batch b = c//4, heads 4g..4g+3 where g = c%4.
Each core computes q,k,v for its 4 heads (over its batch's 2048 tokens),
causal softmax attention in transposed-score layout [k, q] (denominator via
an extra ones-column on v), and the partial output projection over its 256
head-channels. Host sums the 4 partials per batch and adds b_proj.

All matmuls run as float32r (full PE rate at N>=256, ~1e-4 relative
rounding). The 1/sqrt(d) score scale is folded into W_k/b_k on the host.
"""

import os
import sys

for _p in ("/opt/trn_rl_repo", "/opt/pypackages"):
    if os.path.isdir(_p) and _p not in sys.path:
        sys.path.append(_p)

import numpy as np

import concourse.bass as bass
import concourse.tile as tile
import concourse.mybir as mybir
from concourse import bacc
from concourse.bass_utils import run_bass_kernel_spmd

B, T, C = 2, 2048, 1024
H = 16            # total heads
D = 64            # head dim
HPC = 4           # heads per core
CH = HPC * D      # 256 channels per core
N_CORES = 8

f32 = mybir.dt.float32
f32r = mybir.dt.float32r
ts = bass.ts

_COMPILED = None


def _build():
    nc = bacc.Bacc("TRN2", target_bir_lowering=False, debug=False,
                   num_devices=N_CORES)

    xT = nc.dram_tensor("xT", [C, T], f32, kind="ExternalInput").ap()
    wt = nc.dram_tensor("wt", [C, 3 * CH], f32, kind="ExternalInput").ap()
    wpt = nc.dram_tensor("wpt", [CH, C], f32, kind="ExternalInput").ap()
    bqk = nc.dram_tensor("bqk", [128, 4], f32, kind="ExternalInput").ap()
    bvb = nc.dram_tensor("bvb", [128, CH], f32, kind="ExternalInput").ap()
    Sm = nc.dram_tensor("Sm", [128, 1024], f32, kind="ExternalInput").ap()
    out = nc.dram_tensor("out_partial", [T, C], f32, kind="ExternalOutput").ap()

    NT512 = T // 512          # 4   512-token tiles
    NT128 = T // 128          # 16  128-token tiles
    NC128 = C // 128          # 8   128-channel tiles of the contraction

    with tile.TileContext(nc) as tc:
        with tc.tile_pool(name="consts", bufs=1) as consts, \
             tc.tile_pool(name="qkv", bufs=1) as qkv, \
             tc.tile_pool(name="xp", bufs=2) as xp, \
             tc.tile_pool(name="pp", bufs=3) as pp, \
             tc.tile_pool(name="op", bufs=3) as op, \
             tc.tile_pool(name="small", bufs=2) as small:

            # ---- constants ----
            wt_sb = consts.tile([128, NC128, 3 * CH], f32r)
            nc.sync.dma_start(
                wt_sb[:], wt.rearrange("(o p) f -> p o f", p=128).bitcast(f32r))
            wpt_sb = consts.tile([128, 2, C], f32r)
            nc.sync.dma_start(
                wpt_sb[:], wpt.rearrange("(s p) o -> p s o", p=128).bitcast(f32r))
            bqk_sb = consts.tile([128, 4], f32)
            nc.sync.dma_start(bqk_sb[:], bqk)
            bvb_sb = consts.tile([128, CH], f32)
            nc.sync.dma_start(bvb_sb[:], bvb)
            S_sb = consts.tile([128, 1024], f32r)
            nc.sync.dma_start(S_sb[:], Sm.bitcast(f32r))

            ones_f = consts.tile([1, 128], f32)
            nc.vector.memset(ones_f[:], 1.0)
            ones_r = consts.tile([1, 128], f32r)
            nc.vector.tensor_copy(ones_r[:], ones_f[:])
            onecol_f = consts.tile([128, 1], f32)
            nc.vector.memset(onecol_f[:], 1.0)

            # ---- persistent activations ----
            qT = qkv.tile([128, 2, T], f32r)      # [2h*64, slab, t]
            kT = qkv.tile([128, 2, T], f32r)
            vaug = qkv.tile([128, NT128, HPC, D + 1], f32r)  # [t128, ti, h, d|1]
            yT = qkv.tile([128, 2, T], f32r)

            # ones column of vaug
            for h in range(HPC):
                nc.vector.tensor_copy(
                    vaug[:, :, h, D:D + 1],
                    onecol_f[:].to_broadcast([128, NT128, 1]))

            # ================= Phase A: QKV projection =================
            with tc.tile_pool(name="ps_qk", bufs=3, space="PSUM") as ps_qk, \
                 tc.tile_pool(name="ps_v", bufs=3, space="PSUM") as ps_v:
                for ti in range(NT512):
                    xt = xp.tile([128, NC128, 512], f32r)
                    nc.sync.dma_start(
                        xt[:],
                        xT.rearrange("(o p) t -> p o t", p=128)
                          [:, :, ts(ti, 512)].bitcast(f32r))
                    # q,k: produce transposed [f, t] layout
                    for fj in range(4):          # q0 q1 k0 k1
                        ps = ps_qk.tile([128, 512], f32)
                        for ci in range(NC128):
                            nc.tensor.matmul(
                                ps[:], wt_sb[:, ci, ts(fj, 128)], xt[:, ci, :],
                                start=(ci == 0), stop=(ci == NC128 - 1))
                        dest = qT if fj < 2 else kT
                        nc.vector.tensor_add(
                            out=dest[:, fj % 2, ts(ti, 512)], in0=ps[:],
                            in1=bqk_sb[:, fj:fj + 1].to_broadcast([128, 512]))
                    # v: produce [t, ch] layout directly
                    for tj in range(4):
                        pv = ps_v.tile([128, CH], f32)
                        for ci in range(NC128):
                            nc.tensor.matmul(
                                pv[:], xt[:, ci, ts(tj, 128)],
                                wt_sb[:, ci, 512:512 + CH],
                                start=(ci == 0), stop=(ci == NC128 - 1))
                        for h in range(HPC):
                            nc.vector.tensor_add(
                                out=vaug[:, 4 * ti + tj, h, 0:D],
                                in0=pv[:, ts(h, D)],
                                in1=bvb_sb[:, ts(h, D)])

            # ========== Phase B: attention + output projection ==========
            with tc.tile_pool(name="ps_s", bufs=3, space="PSUM") as ps_s, \
                 tc.tile_pool(name="ps_y", bufs=2, space="PSUM") as ps_y, \
                 tc.tile_pool(name="ps_b", bufs=1, space="PSUM") as ps_b, \
                 tc.tile_pool(name="ps_o", bufs=2, space="PSUM") as ps_o:
                for qi in range(NT512):
                    for h in range(HPC):
                        hp, hs = (h % 2) * D, h // 2
                        py = ps_y.tile([D + 1, 512], f32)
                        nk = 4 * qi + 4
                        for ki in range(nk):
                            psc = ps_s.tile([128, 512], f32)
                            nc.tensor.matmul(
                                psc[:],
                                kT[hp:hp + D, hs, ts(ki, 128)],
                                qT[hp:hp + D, hs, ts(qi, 512)],
                                start=True, stop=True)
                            p = pp.tile([128, 512], f32r)
                            nc.scalar.activation(
                                p[:], psc[:], mybir.ActivationFunctionType.Exp)
                            j = ki - 4 * qi
                            if j >= 0:  # diagonal block: causal mask
                                off = 384 - 128 * j
                                nc.vector.tensor_mul(
                                    out=p[:], in0=p[:],
                                    in1=S_sb[:, off:off + 512])
                            nc.tensor.matmul(
                                py[:], vaug[:, ki, h, :], p[:],
                                start=(ki == 0), stop=(ki == nk - 1))
                        # normalize: yT = py[:D] * (1/py[D]) broadcast
                        rec = small.tile([1, 512], f32, tag="rec")
                        nc.vector.reciprocal(rec[:], py[D:D + 1, :])
                        rec_r = small.tile([1, 512], f32r, tag="rec_r")
                        nc.vector.tensor_copy(rec_r[:], rec[:])
                        pb = ps_b.tile([D, 512], f32)
                        nc.tensor.matmul(pb[:], ones_r[:, :D], rec_r[:],
                                         start=True, stop=True)
                        bc = small.tile([D, 512], f32, tag="bc")
                        nc.scalar.activation(
                            bc[:], pb[:], mybir.ActivationFunctionType.Copy)
                        nc.vector.tensor_mul(
                            out=yT[hp:hp + D, hs, ts(qi, 512)],
                            in0=py[0:D, :], in1=bc[:])
                    # output projection for this 512-token stripe
                    for tj in range(4):
                        tg = 4 * qi + tj
                        for oi in range(2):
                            po = ps_o.tile([128, 512], f32)
                            for s in range(2):
                                nc.tensor.matmul(
                                    po[:], yT[:, s, ts(tg, 128)],
                                    wpt_sb[:, s, ts(oi, 512)],
                                    start=(s == 0), stop=(s == 1))
                            ot = op.tile([128, 512], f32)
                            nc.scalar.activation(
                                ot[:], po[:],
                                mybir.ActivationFunctionType.Copy)
                            nc.sync.dma_start(
                                out[ts(tg, 128), ts(oi, 512)], ot[:])

    nc.compile()
    return nc


def _get_compiled():
    global _COMPILED
    if _COMPILED is None:
        _COMPILED = _build()
    return _COMPILED


def _host_prep(x, W_attn, b_attn, W_proj, b_proj):
    scale = 1.0 / np.sqrt(np.float32(D))
    xTb = [np.ascontiguousarray(x[b].T).astype(np.float32) for b in range(B)]
    Sm = (np.arange(1024, dtype=np.int32)[None, :]
          >= (np.arange(128, dtype=np.int32)[:, None] + 384)).astype(np.float32)
    in_maps = []
    for c in range(N_CORES):
        b, g = divmod(c, 4)
        ch = slice(CH * g, CH * (g + 1))
        Wq = W_attn[ch]
        Wk = W_attn[C:][ch] * scale
        Wv = W_attn[2 * C:][ch]
        wt_c = np.ascontiguousarray(
            np.concatenate([Wq, Wk, Wv], axis=0).T).astype(np.float32)
        bq = b_attn[ch]
        bk = b_attn[C:][ch] * scale
        bv = b_attn[2 * C:][ch]
        bqk_c = np.ascontiguousarray(
            np.concatenate([bq, bk]).reshape(4, 128).T).astype(np.float32)
        bvb_c = np.ascontiguousarray(
            np.broadcast_to(bv[None, :], (128, CH))).astype(np.float32)
        wpt_c = np.ascontiguousarray(W_proj[:, ch].T).astype(np.float32)
        in_maps.append({
            "xT": xTb[b],
            "wt": wt_c,
            "wpt": wpt_c,
            "bqk": bqk_c,
            "bvb": bvb_c,
            "Sm": Sm,
        })
    return in_maps


def kernel(x, W_attn, b_attn, W_proj, b_proj):
    x = np.asarray(x, dtype=np.float32)
    W_attn = np.asarray(W_attn, dtype=np.float32)
    b_attn = np.asarray(b_attn, dtype=np.float32)
    W_proj = np.asarray(W_proj, dtype=np.float32)
    b_proj = np.asarray(b_proj, dtype=np.float32)

    nc = _get_compiled()
    in_maps = _host_prep(x, W_attn, b_attn, W_proj, b_proj)
    res = run_bass_kernel_spmd(nc, in_maps, core_ids=list(range(N_CORES)))

    out = np.empty((B, T, C), dtype=np.float32)
    for b in range(B):
        acc = res.results[4 * b]["out_partial"].copy()
        for g in range(1, 4):
            acc += res.results[4 * b + g]["out_partial"]
        out[b] = acc + b_proj
    return out
